# revision 30
# baseline (speedup 1.0000x reference)
"""Causal multi-head attention with RoPE on 8 Trainium2 NeuronCores.

Reference computation (fp32):
    qkv = x @ Wqkv.T ; split q,k,v ; heads 16 x 64 ; interleaved-pair RoPE on
    q,k ; causal softmax(q k^T / 8) @ v ; concat heads ; out @ Wout.T

Sharding: core c -> batch b=c//2, head-group g=c%2 (heads 8g..8g+8).
Each core computes a [2048, 1024] partial of the output projection for its
batch (contraction over its 512 head-dims); host sums core pairs.

Kernel-internal layout tricks:
  - Wqkv rows per head are permuted evens-then-odds so RoPE becomes
    block-wise (no interleaving on device). The same permutation applied to
    q and k leaves q.k^T invariant.
  - Scores are computed transposed (S^T[k, q]) so the PV matmul needs no
    transposes; PV uses a ones-augmented V (M=65) so row 64 of the PV psum
    accumulates the softmax denominator for free.
  - Causal masks are added into the scores psum by an accumulating
    identity @ mask matmul on the PE (keeps DVE free, keeps PE warm).
  - Denominators are transposed via the PE, reciprocated in fp32, and
    broadcast across partitions with a ones @ diag(recip) matmul; the
    division is one elementwise multiply per output tile.

Matmul dtype MM_DT (env): bfloat16 (default, host pre-rounds inputs),
float32r, or float32. The softmax denominator / division chain is fp32
in all modes.
"""

import math
import os
import sys

import numpy as np

sys.path.insert(0, "/opt/trn_rl_repo")

import concourse.bass as bass  # noqa: E402,F401  (re-exported for tooling)
import concourse.mybir as mybir  # noqa: E402
from concourse import bacc, tile  # noqa: E402
from concourse.masks import make_identity  # noqa: E402

D_MODEL = 1024
NUM_HEADS = 16
DH = 64
S = 2048
B = 4
THETA = 10000.0
P = 128
N_CORES = 8
F = 512  # free-dim chunk
N_SC = S // F  # 4 s-chunks
N_QT = S // P  # 16 q-tiles of 128
HPAIRS = 4  # head pairs per core
NEG = -1.0e30

MM_DT = getattr(mybir.dt, os.environ.get("MM_DT", "bfloat16"))


def build_program(debug: bool = False):
    """Build the single-core SPMD program (identical on all 8 cores)."""
    nc = bacc.Bacc("TRN2", target_bir_lowering=False, debug=debug,
                   enable_asserts=debug)
    f32 = mybir.dt.float32
    cdt = MM_DT

    xt_d = nc.dram_tensor("xt", [D_MODEL, S], cdt, kind="ExternalInput")
    wq_d = nc.dram_tensor("wqkv", [D_MODEL, 12 * P], cdt, kind="ExternalInput")
    wo_d = nc.dram_tensor("wout", [4 * P, D_MODEL], cdt, kind="ExternalInput")
    cos_d = nc.dram_tensor("costab", [P, S], cdt, kind="ExternalInput")
    sinw_d = nc.dram_tensor("sinswt", [P, S], cdt, kind="ExternalInput")
    mask_d = nc.dram_tensor("masks", [P, P], cdt, kind="ExternalInput")
    out_d = nc.dram_tensor("out", [S, D_MODEL], f32, kind="ExternalOutput")

    xt_r = xt_d.ap().rearrange("(dc p) s -> p dc s", p=P)  # [128, 8, 2048]
    wq_r = wq_d.ap().rearrange("(dc p) n -> p dc n", p=P)  # [128, 8, 1536]
    wo_r = wo_d.ap().rearrange("(hp p) e -> p hp e", p=P)  # [128, 4, 1024]


    with tile.TileContext(nc) as tc:
        with (
            tc.tile_pool(name="const", bufs=1) as const,
            tc.tile_pool(name="wq", bufs=2) as wqp,
            tc.tile_pool(name="qkv", bufs=2) as qkvp,
            tc.tile_pool(name="tmp", bufs=3) as tmpp,
            tc.tile_pool(name="outt", bufs=1) as outtp,
            tc.tile_pool(name="exp", bufs=8) as expp,
            tc.tile_pool(name="fin", bufs=3) as finp,
            tc.tile_pool(name="small", bufs=2) as smallp,
            tc.tile_pool(name="psb", bufs=3, space="PSUM") as psb,
        ):
            # ---- constants ----
            ident = const.tile([P, P], f32)
            make_identity(nc, ident)
            identc = const.tile([P, P], cdt)
            nc.vector.tensor_copy(identc[:], ident[:])
            ones64 = const.tile([P, 64], f32)
            nc.vector.memset(ones64[:], 1.0)
            rdt = mybir.dt.float32r if MM_DT != mybir.dt.float32 else f32
            ones64r = const.tile([P, 64], rdt)
            nc.vector.tensor_copy(ones64r[:], ones64[:])
            ident4 = const.tile([P, 4, P], rdt)
            for _i4 in range(4):
                nc.vector.tensor_copy(ident4[:, _i4, :], ident[:])
            cost = const.tile([P, S], cdt)
            nc.sync.dma_start(cost[:], cos_d.ap())
            sinw = const.tile([P, S], cdt)
            nc.sync.dma_start(sinw[:], sinw_d.ap())
            maskt = const.tile([P, P], cdt)
            nc.sync.dma_start(maskt[:], mask_d.ap())
            woutt = const.tile([P, 4, D_MODEL], cdt)
            nc.sync.dma_start(woutt[:], wo_r)
            # x^T resident: [128, dchunk, s]
            xts = const.tile([P, 8, S], cdt)
            nc.sync.dma_start(xts[:], xt_r)
            # attention output (d-major), all 4 head pairs: rows=[hA|hB] dims
            outt = outtp.tile([P, HPAIRS, S], cdt)
            # softmax denominators, one tile per head pair: head A on row 0,
            # head B on row 32 (engine APs need 32-aligned start partitions)
            rs_hp = []
            for _hp in range(HPAIRS):
                _t = const.tile([P, S], f32, name=f"rs{_hp}")
                nc.vector.memset(_t[:], 1.0)
                rs_hp.append(_t)
            # reciprocals, transposed: [q-within-tile, qtile, hpair*2+h2]
            rcp = const.tile([P, N_QT, 8], f32)

            for hp in range(HPAIRS):
                _sid_qkv = nc.enter_named_scope(f"qkv{hp}", False)[0]
                whp = wqp.tile([P, 8, 3 * P], cdt)
                nc.sync.dma_start(whp[:], wq_r[:, :, hp * 3 * P:(hp + 1) * 3 * P])
                q_rot = qkvp.tile([P, S], cdt, tag="q_rot")
                k_rot = qkvp.tile([P, S], cdt, tag="k_rot")
                # V s-major + ones cols: [s-part, ktile, (vA|1|vB|1)]
                v_sb = qkvp.tile([P, N_QT, 130], cdt, tag="v_sb")
                nc.vector.tensor_copy(v_sb[:, :, 64:65], ones64[:, 0:N_QT, None])
                nc.vector.tensor_copy(v_sb[:, :, 129:130],
                                      ones64[:, 0:N_QT, None])

                for sc in range(N_SC):
                    sl = slice(sc * F, (sc + 1) * F)
                    # q and k groups (d-major); psum evac to sbuf via ACT,
                    # then RoPE on DVE in the compute dtype (2x mode for bf16)
                    for gi, dst in ((0, q_rot), (1, k_rot)):
                        psw_ = psb.tile([P, 2 * F], f32, tag="s1", bufs=3,
                                        name="psw")
                        ps = psw_[:, 0:F]
                        for dc in range(8):
                            nc.tensor.matmul(
                                ps,
                                whp[:, dc, gi * P:(gi + 1) * P],
                                xts[:, dc, sl],
                                start=(dc == 0), stop=(dc == 7),
                            )
                        qk = tmpp.tile([P, F], cdt, tag="qk_sb")
                        nc.scalar.copy(qk[:], ps)
                        # rot = qk*cos + swap_within_head(qk)*sins
                        tcs = tmpp.tile([P, F], cdt, tag="ropetmp")
                        nc.vector.tensor_tensor(tcs[:], qk[:], cost[:, sl],
                                                mybir.AluOpType.mult)
                        for h2 in (0, 64):
                            nc.vector.tensor_tensor(
                                dst[h2:h2 + 32, sl], qk[h2 + 32:h2 + 64, :],
                                sinw[h2 + 32:h2 + 64, sl],
                                mybir.AluOpType.mult)
                            nc.vector.tensor_tensor(
                                dst[h2 + 32:h2 + 64, sl], qk[h2:h2 + 32, :],
                                sinw[h2:h2 + 32, sl], mybir.AluOpType.mult)
                        nc.vector.tensor_tensor(dst[:, sl], dst[:, sl],
                                                tcs[:], mybir.AluOpType.add)
                    # v group: d-major matmul, then PE-transpose to s-major
                    psw_ = psb.tile([P, 2 * F], f32, tag="s1", bufs=3,
                                    name="pswv")
                    ps = psw_[:, 0:F]
                    for dc in range(8):
                        nc.tensor.matmul(
                            ps, whp[:, dc, 2 * P:3 * P],
                            xts[:, dc, sl], start=(dc == 0), stop=(dc == 7),
                        )
                    vdm = tmpp.tile([P, F], cdt, tag="vdm")
                    nc.scalar.copy(vdm[:], ps)
                    for j in range(4):
                        kt = sc * 4 + j
                        pt = psb.tile([P, F], cdt, tag="s1", bufs=3, name="pt")
                        nc.tensor.transpose(pt[:, 0:P],
                                            vdm[:, j * P:(j + 1) * P],
                                            identc[:])
                        nc.vector.tensor_copy(v_sb[:, kt, 0:64], pt[:, 0:64])
                        nc.vector.tensor_copy(v_sb[:, kt, 65:129],
                                              pt[:, 64:128])

                nc.leave_named_scope(f"qkv{hp}", _sid_qkv, False)
                # ---- causal attention for this head pair ----
                _sid_attn = nc.enter_named_scope(f"attn{hp}", False)[0]
                for qc in range(N_SC):
                    qsl = slice(qc * F, (qc + 1) * F)
                    po = [psb.tile([P, F], f32, tag="po", bufs=2,
                                   name=f"po{h2}")
                          for h2 in range(2)]
                    nkt = 4 * qc + 4
                    LOOKAHEAD = 2

                    def emit_scores(kt):
                        lo = max(0, (kt - 4 * qc) * P)
                        diag = kt >= 4 * qc
                        # one psum tile holds BOTH heads' scores side by side:
                        # the two matmuls allocate together and issue
                        # back-to-back, so their disjoint row groups
                        # (0:64 / 64:128) run concurrently on the PE
                        spb = psb.tile([P, 2 * F], f32, tag="s1", bufs=3,
                                       name="spb")
                        sps = [spb[:, 0:F], spb[:, F:2 * F]]
                        for h2 in (0, 1):
                            base = 64 * h2
                            nc.tensor.matmul(
                                sps[h2][:, lo:F],
                                k_rot[base:base + 64, kt * P:(kt + 1) * P],
                                q_rot[base:base + 64,
                                      qc * F + lo:(qc + 1) * F],
                                start=True, stop=not diag,
                                skip_group_check=diag,
                            )
                        if diag:
                            # strictly-upper triangle mask on the 128-wide
                            # diagonal block only
                            for h2 in (0, 1):
                                nc.tensor.matmul(
                                    sps[h2][:, lo:lo + P],
                                    identc[:], maskt[:],
                                    start=False, stop=True,
                                    skip_group_check=True,
                                )
                        return sps

                    pend = {kt: emit_scores(kt)
                            for kt in range(min(LOOKAHEAD, nkt))}
                    for kt in range(nkt):
                        lo = max(0, (kt - 4 * qc) * P)
                        if kt + LOOKAHEAD < nkt:
                            pend[kt + LOOKAHEAD] = emit_scores(kt + LOOKAHEAD)
                        sps = pend.pop(kt)
                        exs = []
                        for h2 in (0, 1):
                            ex = expp.tile([P, F], cdt)
                            if lo > 0:
                                nc.gpsimd.memset(ex[:, 0:lo], 0.0)
                            nc.scalar.activation(
                                ex[:, lo:F], sps[h2][:, lo:F],
                                mybir.ActivationFunctionType.Exp,
                                scale=1.0 / math.sqrt(DH))
                            exs.append(ex)
                        for h2 in (0, 1):
                            nc.tensor.matmul(
                                po[h2][0:65, :],
                                v_sb[:, kt, 65 * h2:65 * h2 + 65],
                                exs[h2][:],
                                start=(kt == 0), stop=(kt == nkt - 1),
                            )
                    for h2 in (0, 1):
                        nc.vector.tensor_copy(
                            outt[64 * h2:64 * h2 + 64, hp, qsl],
                            po[h2][0:64, :])
                        nc.vector.tensor_copy(rs_hp[hp][32 * h2:32 * h2 + 1,
                                                        qsl],
                                              po[h2][64:65, :])

                nc.leave_named_scope(f"attn{hp}", _sid_attn, False)
                _sid_div = nc.enter_named_scope(f"div{hp}", False)[0]
                # denominators for this head pair: transpose+reciprocal,
                # then broadcast and divide (overlaps next pair's attention)
                for j in range(N_QT):
                    pt = psb.tile([P, F], f32, tag="s1", bufs=3, name="ptr")
                    nc.tensor.transpose(pt[:, 0:P],
                                        rs_hp[hp][:, j * P:(j + 1) * P],
                                        ident[:])
                    for h2 in (0, 1):
                        nc.vector.reciprocal(rcp[:, j, 2 * hp + h2:
                                                 2 * hp + h2 + 1],
                                             pt[:, 32 * h2:32 * h2 + 1])
                for qc in range(N_SC):
                    qsl = slice(qc * F, (qc + 1) * F)
                    for h2 in (0, 1):
                        h = 2 * hp + h2
                        diag4 = smallp.tile([P, 4, P], rdt, tag="diag", bufs=4)
                        nc.vector.tensor_tensor(
                            diag4[:], ident4[:],
                            rcp[:, 4 * qc:4 * qc + 4, h:h + 1].to_broadcast(
                                (P, 4, P)),
                            mybir.AluOpType.mult)
                        pbcw = psb.tile([P, 2 * F], f32, tag="s1", bufs=3,
                                        name="pbcw")
                        pbc = pbcw[:, 0:F]
                        nc.tensor.matmul(pbc[0:64, :], ones64r[:, 0:64],
                                         diag4[:], start=True, stop=True)
                        nc.vector.tensor_tensor(
                            outt[64 * h2:64 * h2 + 64, hp, qsl],
                            outt[64 * h2:64 * h2 + 64, hp, qsl],
                            pbc[0:64, :], mybir.AluOpType.mult)

                nc.leave_named_scope(f"div{hp}", _sid_div, False)
            # ---- output projection: natural [s, e] partial ----
            _sid_proj = nc.enter_named_scope("proj", False)[0]
            for ec in range(2):
                esl = slice(ec * F, (ec + 1) * F)
                for st in range(N_QT):
                    pfw_ = psb.tile([P, 2 * F], f32, tag="s1", bufs=3,
                                    name="pfw")
                    pf = pfw_[:, 0:F]
                    for hp in range(HPAIRS):
                        nc.tensor.matmul(
                            pf, outt[:, hp, st * P:(st + 1) * P],
                            woutt[:, hp, esl],
                            start=(hp == 0), stop=(hp == 3),
                        )
                    fo = finp.tile([P, F], f32)
                    nc.scalar.copy(fo[:], pf)
                    nc.sync.dma_start(
                        out_d.ap()[st * P:(st + 1) * P, esl], fo[:])
            nc.leave_named_scope("proj", _sid_proj, False)

    nc.compile()
    return nc


def _rope_tables():
    k = np.arange(DH // 2, dtype=np.float64)
    invf = THETA ** (-2.0 * k / DH)
    pos = np.arange(S, dtype=np.float64)
    ang = invf[:, None] * pos[None, :]  # [32, S]
    cos32 = np.cos(ang)
    sin32 = np.sin(ang)
    cos = np.tile(cos32, (4, 1)).astype(np.float32)          # [128, S]
    sins = np.concatenate([-sin32, sin32, -sin32, sin32], 0).astype(np.float32)
    return cos, sins


def _masks():
    i = np.arange(P)[:, None]
    j = np.arange(P)[None, :]
    return np.where(i > j, np.float32(NEG), np.float32(0.0))


def _np_dt():
    if MM_DT == mybir.dt.bfloat16:
        import ml_dtypes
        return np.dtype(ml_dtypes.bfloat16)
    return np.dtype(np.float32)


def host_inputs(x, Wqkv, Wout, core):
    """Per-core input dict (cast to the compute dtype on host)."""
    ndt = _np_dt()
    b, g = core // 2, core % 2
    xt = np.ascontiguousarray(x[b].T).astype(ndt)  # [1024, 2048]
    perm = np.concatenate([np.arange(0, DH, 2), np.arange(1, DH, 2)])
    blocks = []
    for hp in range(HPAIRS):
        hA = 8 * g + 2 * hp
        for off, do_perm in ((0, True), (D_MODEL, True), (2 * D_MODEL, False)):
            for h in (hA, hA + 1):
                rows = Wqkv[off + h * DH: off + (h + 1) * DH]
                if do_perm:
                    rows = rows[perm]
                blocks.append(rows)
    wq = np.ascontiguousarray(np.concatenate(blocks, 0).T).astype(ndt)
    wo = np.ascontiguousarray(Wout[:, 512 * g:512 * (g + 1)].T).astype(ndt)
    cos, sins = _rope_tables()
    return {"xt": xt, "wqkv": wq, "wout": wo,
            "costab": cos.astype(ndt), "sinswt": (-sins).astype(ndt),
            "masks": _masks().astype(ndt)}


_CACHE = {}


def kernel(x, Wqkv, Wout):
    from concourse.bass_utils import run_bass_kernel_spmd

    x = np.asarray(x, dtype=np.float32)
    Wqkv = np.asarray(Wqkv, dtype=np.float32)
    Wout = np.asarray(Wout, dtype=np.float32)

    if "nc" not in _CACHE:
        _CACHE["nc"] = build_program(debug=False)
    nc = _CACHE["nc"]

    in_maps = [host_inputs(x, Wqkv, Wout, c) for c in range(N_CORES)]
    res = run_bass_kernel_spmd(nc, in_maps, list(range(N_CORES))).results
    out = np.empty((B, S, D_MODEL), dtype=np.float32)
    for b in range(B):
        out[b] = res[2 * b]["out"] + res[2 * b + 1]["out"]
    return out


# revision 31
# speedup vs baseline: 1.2490x; 1.2490x over previous
"""Causal multi-head attention with RoPE on 8 Trainium2 NeuronCores.

Reference computation (fp32):
    qkv = x @ Wqkv.T ; split q,k,v ; heads 16 x 64 ; interleaved-pair RoPE on
    q,k ; causal softmax(q k^T / 8) @ v ; concat heads ; out @ Wout.T

Sharding: core c -> batch b=c//2, head-group g=c%2 (heads 8g..8g+8).
Each core computes a [2048, 1024] partial of the output projection for its
batch (contraction over its 512 head-dims); host sums core pairs.

Kernel-internal layout tricks:
  - Wqkv rows per head are permuted evens-then-odds so RoPE becomes
    block-wise (no interleaving on device). The same permutation applied to
    q and k leaves q.k^T invariant.
  - Scores are computed transposed (S^T[k, q]) so the PV matmul needs no
    transposes; PV uses a ones-augmented V (M=65) so row 64 of the PV psum
    accumulates the softmax denominator for free.
  - Causal masks are added into the scores psum by an accumulating
    identity @ mask matmul on the PE (keeps DVE free, keeps PE warm).
  - Denominators are transposed via the PE, reciprocated in fp32, and
    broadcast across partitions with a ones @ diag(recip) matmul; the
    division is one elementwise multiply per output tile.

Matmul dtype MM_DT (env): bfloat16 (default, host pre-rounds inputs),
float32r, or float32. The softmax denominator / division chain is fp32
in all modes.
"""

import math
import os
import sys

import numpy as np

sys.path.insert(0, "/opt/trn_rl_repo")

import concourse.bass as bass  # noqa: E402,F401  (re-exported for tooling)
import concourse.mybir as mybir  # noqa: E402
from concourse import bacc, tile  # noqa: E402
from concourse.masks import make_identity  # noqa: E402

D_MODEL = 1024
NUM_HEADS = 16
DH = 64
S = 2048
B = 4
THETA = 10000.0
P = 128
N_CORES = 8
F = 512  # free-dim chunk
N_SC = S // F  # 4 s-chunks
N_QT = S // P  # 16 q-tiles of 128
HPAIRS = 4  # head pairs per core
NEG = -1.0e30

MM_DT = getattr(mybir.dt, os.environ.get("MM_DT", "bfloat16"))


def build_program(debug: bool = False):
    """Build the single-core SPMD program (identical on all 8 cores)."""
    nc = bacc.Bacc("TRN2", target_bir_lowering=False, debug=debug,
                   enable_asserts=debug)
    f32 = mybir.dt.float32
    cdt = MM_DT

    xt_d = nc.dram_tensor("xt", [D_MODEL, S], cdt, kind="ExternalInput")
    wq_d = nc.dram_tensor("wqkv", [D_MODEL, 12 * P], cdt, kind="ExternalInput")
    wo_d = nc.dram_tensor("wout", [4 * P, D_MODEL], cdt, kind="ExternalInput")
    cos_d = nc.dram_tensor("costab", [P, S], cdt, kind="ExternalInput")
    sinw_d = nc.dram_tensor("sinswt", [P, S], cdt, kind="ExternalInput")
    mask_d = nc.dram_tensor("masks", [P, P], cdt, kind="ExternalInput")
    out_d = nc.dram_tensor("out", [S, D_MODEL], f32, kind="ExternalOutput")

    xt_r = xt_d.ap().rearrange("(dc p) s -> p dc s", p=P)  # [128, 8, 2048]
    wq_r = wq_d.ap().rearrange("(dc p) n -> p dc n", p=P)  # [128, 8, 1536]
    wo_r = wo_d.ap().rearrange("(hp p) e -> p hp e", p=P)  # [128, 4, 1024]


    with tile.TileContext(nc) as tc:
        with (
            tc.tile_pool(name="const", bufs=1) as const,
            tc.tile_pool(name="wq", bufs=2) as wqp,
            tc.tile_pool(name="qkv", bufs=2) as qkvp,
            tc.tile_pool(name="tmp", bufs=3) as tmpp,
            tc.tile_pool(name="outt", bufs=1) as outtp,
            tc.tile_pool(name="exp", bufs=8) as expp,
            tc.tile_pool(name="fin", bufs=3) as finp,
            tc.tile_pool(name="small", bufs=2) as smallp,
            tc.tile_pool(name="psb", bufs=4, space="PSUM") as psb,
            tc.tile_pool(name="pss", bufs=2, space="PSUM") as pss,
        ):
            # ---- constants ----
            ident = const.tile([P, P], f32)
            make_identity(nc, ident)
            identc = const.tile([P, P], cdt)
            nc.vector.tensor_copy(identc[:], ident[:])
            ones64 = const.tile([P, 64], f32)
            nc.vector.memset(ones64[:], 1.0)
            rdt = mybir.dt.float32r if MM_DT != mybir.dt.float32 else f32
            ones64r = const.tile([P, 64], rdt)
            nc.vector.tensor_copy(ones64r[:], ones64[:])
            ident4 = const.tile([P, 4, P], rdt)
            for _i4 in range(4):
                nc.vector.tensor_copy(ident4[:, _i4, :], ident[:])
            cost = const.tile([P, S], cdt)
            nc.sync.dma_start(cost[:], cos_d.ap())
            sinw = const.tile([P, S], cdt)
            nc.sync.dma_start(sinw[:], sinw_d.ap())
            maskt = const.tile([P, P], cdt)
            nc.sync.dma_start(maskt[:], mask_d.ap())
            woutt = const.tile([P, 4, D_MODEL], cdt)
            nc.sync.dma_start(woutt[:], wo_r)
            # x^T resident: [128, dchunk, s]
            xts = const.tile([P, 8, S], cdt)
            nc.sync.dma_start(xts[:], xt_r)
            # attention output (d-major), all 4 head pairs: rows=[hA|hB] dims
            outt = outtp.tile([P, HPAIRS, S], cdt)
            # softmax denominators, one tile per head pair: head A on row 0,
            # head B on row 32 (engine APs need 32-aligned start partitions)
            rs_hp = []
            for _hp in range(HPAIRS):
                _t = const.tile([P, S], f32, name=f"rs{_hp}")
                nc.vector.memset(_t[:], 1.0)
                rs_hp.append(_t)
            # reciprocals, transposed: [q-within-tile, qtile, hpair*2+h2]
            rcp = const.tile([P, N_QT, 8], f32)

            for hp in range(HPAIRS):
                _sid_qkv = nc.enter_named_scope(f"qkv{hp}", False)[0]
                whp = wqp.tile([P, 8, 3 * P], cdt)
                nc.sync.dma_start(whp[:], wq_r[:, :, hp * 3 * P:(hp + 1) * 3 * P])
                q_rot = qkvp.tile([P, S], cdt, tag="q_rot")
                k_rot = qkvp.tile([P, S], cdt, tag="k_rot")
                # V s-major + ones cols: [s-part, ktile, (vA|1|vB|1)]
                v_sb = qkvp.tile([P, N_QT, 130], cdt, tag="v_sb")
                nc.vector.tensor_copy(v_sb[:, :, 64:65], ones64[:, 0:N_QT, None])
                nc.vector.tensor_copy(v_sb[:, :, 129:130],
                                      ones64[:, 0:N_QT, None])

                for sc in range(N_SC):
                    sl = slice(sc * F, (sc + 1) * F)
                    # q and k groups (d-major); psum evac to sbuf via ACT,
                    # then RoPE on DVE in the compute dtype (2x mode for bf16)
                    for gi, dst in ((0, q_rot), (1, k_rot)):
                        ps = psb.tile([P, F], f32, tag="s1", bufs=4,
                                      name="psw")
                        for dc in range(8):
                            nc.tensor.matmul(
                                ps,
                                whp[:, dc, gi * P:(gi + 1) * P],
                                xts[:, dc, sl],
                                start=(dc == 0), stop=(dc == 7),
                            )
                        qk = tmpp.tile([P, F], cdt, tag="qk_sb")
                        nc.scalar.copy(qk[:], ps)
                        # rot = qk*cos + swap_within_head(qk)*sins
                        tcs = tmpp.tile([P, F], cdt, tag="ropetmp")
                        nc.vector.tensor_tensor(tcs[:], qk[:], cost[:, sl],
                                                mybir.AluOpType.mult)
                        for h2 in (0, 64):
                            nc.vector.tensor_tensor(
                                dst[h2:h2 + 32, sl], qk[h2 + 32:h2 + 64, :],
                                sinw[h2 + 32:h2 + 64, sl],
                                mybir.AluOpType.mult)
                            nc.vector.tensor_tensor(
                                dst[h2 + 32:h2 + 64, sl], qk[h2:h2 + 32, :],
                                sinw[h2:h2 + 32, sl], mybir.AluOpType.mult)
                        nc.vector.tensor_tensor(dst[:, sl], dst[:, sl],
                                                tcs[:], mybir.AluOpType.add)
                    # v group: d-major matmul, then PE-transpose to s-major
                    ps = psb.tile([P, F], f32, tag="s1", bufs=4,
                                  name="pswv")
                    for dc in range(8):
                        nc.tensor.matmul(
                            ps, whp[:, dc, 2 * P:3 * P],
                            xts[:, dc, sl], start=(dc == 0), stop=(dc == 7),
                        )
                    vdm = tmpp.tile([P, F], cdt, tag="vdm")
                    nc.scalar.copy(vdm[:], ps)
                    for j in range(4):
                        kt = sc * 4 + j
                        pt = pss.tile([P, F], cdt, tag="small", name="pt")
                        nc.tensor.transpose(pt[:, 0:P],
                                            vdm[:, j * P:(j + 1) * P],
                                            identc[:])
                        nc.vector.tensor_copy(v_sb[:, kt, 0:64], pt[:, 0:64])
                        nc.vector.tensor_copy(v_sb[:, kt, 65:129],
                                              pt[:, 64:128])

                nc.leave_named_scope(f"qkv{hp}", _sid_qkv, False)
                # ---- causal attention for this head pair ----
                _sid_attn = nc.enter_named_scope(f"attn{hp}", False)[0]
                for qc in range(N_SC):
                    qsl = slice(qc * F, (qc + 1) * F)
                    po = [psb.tile([P, F], f32, tag="po", bufs=2,
                                   name=f"po{h2}")
                          for h2 in range(2)]
                    nkt = 4 * qc + 4
                    LOOKAHEAD = 2

                    def emit_scores(kt):
                        lo = max(0, (kt - 4 * qc) * P)
                        diag = kt >= 4 * qc
                        sps = []
                        # head A and B score matmuls adjacent: disjoint row
                        # groups (0:64 / 64:128) run concurrently on the PE
                        for h2 in (0, 1):
                            sp = psb.tile([P, F], f32, tag="s1", bufs=4,
                                          name=f"s1_{h2}")
                            sps.append(sp)
                            base = 64 * h2
                            nc.tensor.matmul(
                                sp[:, lo:F],
                                k_rot[base:base + 64, kt * P:(kt + 1) * P],
                                q_rot[base:base + 64,
                                      qc * F + lo:(qc + 1) * F],
                                start=True, stop=not diag,
                                skip_group_check=diag,
                            )
                        if diag:
                            # strictly-upper triangle mask on the 128-wide
                            # diagonal block only
                            for h2 in (0, 1):
                                nc.tensor.matmul(
                                    sps[h2][:, lo:lo + P],
                                    identc[:], maskt[:],
                                    start=False, stop=True,
                                    skip_group_check=True,
                                )
                        return sps

                    pend = {kt: emit_scores(kt)
                            for kt in range(min(LOOKAHEAD, nkt))}
                    for kt in range(nkt):
                        lo = max(0, (kt - 4 * qc) * P)
                        if kt + LOOKAHEAD < nkt:
                            pend[kt + LOOKAHEAD] = emit_scores(kt + LOOKAHEAD)
                        sps = pend.pop(kt)
                        exs = []
                        for h2 in (0, 1):
                            ex = expp.tile([P, F], cdt)
                            if lo > 0:
                                nc.gpsimd.memset(ex[:, 0:lo], 0.0)
                            nc.scalar.activation(
                                ex[:, lo:F], sps[h2][:, lo:F],
                                mybir.ActivationFunctionType.Exp,
                                scale=1.0 / math.sqrt(DH))
                            exs.append(ex)
                        for h2 in (0, 1):
                            nc.tensor.matmul(
                                po[h2][0:65, :],
                                v_sb[:, kt, 65 * h2:65 * h2 + 65],
                                exs[h2][:],
                                start=(kt == 0), stop=(kt == nkt - 1),
                            )
                    for h2 in (0, 1):
                        nc.vector.tensor_copy(
                            outt[64 * h2:64 * h2 + 64, hp, qsl],
                            po[h2][0:64, :])
                        nc.vector.tensor_copy(rs_hp[hp][32 * h2:32 * h2 + 1,
                                                        qsl],
                                              po[h2][64:65, :])

                nc.leave_named_scope(f"attn{hp}", _sid_attn, False)
                _sid_div = nc.enter_named_scope(f"div{hp}", False)[0]
                # denominators for this head pair: transpose+reciprocal,
                # then broadcast and divide (overlaps next pair's attention)
                for j in range(N_QT):
                    pt = pss.tile([P, F], f32, tag="small", name="ptr")
                    nc.tensor.transpose(pt[:, 0:P],
                                        rs_hp[hp][:, j * P:(j + 1) * P],
                                        ident[:])
                    for h2 in (0, 1):
                        nc.vector.reciprocal(rcp[:, j, 2 * hp + h2:
                                                 2 * hp + h2 + 1],
                                             pt[:, 32 * h2:32 * h2 + 1])
                for qc in range(N_SC):
                    qsl = slice(qc * F, (qc + 1) * F)
                    for h2 in (0, 1):
                        h = 2 * hp + h2
                        diag4 = smallp.tile([P, 4, P], rdt, tag="diag", bufs=4)
                        nc.vector.tensor_tensor(
                            diag4[:], ident4[:],
                            rcp[:, 4 * qc:4 * qc + 4, h:h + 1].to_broadcast(
                                (P, 4, P)),
                            mybir.AluOpType.mult)
                        pbcw = pss.tile([P, F], f32, tag="small",
                                        name="pbcw")
                        pbc = pbcw[:, 0:F]
                        nc.tensor.matmul(pbc[0:64, :], ones64r[:, 0:64],
                                         diag4[:], start=True, stop=True)
                        nc.vector.tensor_tensor(
                            outt[64 * h2:64 * h2 + 64, hp, qsl],
                            outt[64 * h2:64 * h2 + 64, hp, qsl],
                            pbc[0:64, :], mybir.AluOpType.mult)

                nc.leave_named_scope(f"div{hp}", _sid_div, False)
            # ---- output projection: natural [s, e] partial ----
            _sid_proj = nc.enter_named_scope("proj", False)[0]
            for ec in range(2):
                esl = slice(ec * F, (ec + 1) * F)
                for st in range(N_QT):
                    pf = psb.tile([P, F], f32, tag="s1", bufs=4,
                                  name="pfw")
                    for hp in range(HPAIRS):
                        nc.tensor.matmul(
                            pf, outt[:, hp, st * P:(st + 1) * P],
                            woutt[:, hp, esl],
                            start=(hp == 0), stop=(hp == 3),
                        )
                    fo = finp.tile([P, F], f32)
                    nc.scalar.copy(fo[:], pf)
                    nc.sync.dma_start(
                        out_d.ap()[st * P:(st + 1) * P, esl], fo[:])
            nc.leave_named_scope("proj", _sid_proj, False)

    nc.compile()
    return nc


def _rope_tables():
    k = np.arange(DH // 2, dtype=np.float64)
    invf = THETA ** (-2.0 * k / DH)
    pos = np.arange(S, dtype=np.float64)
    ang = invf[:, None] * pos[None, :]  # [32, S]
    cos32 = np.cos(ang)
    sin32 = np.sin(ang)
    cos = np.tile(cos32, (4, 1)).astype(np.float32)          # [128, S]
    sins = np.concatenate([-sin32, sin32, -sin32, sin32], 0).astype(np.float32)
    return cos, sins


def _masks():
    i = np.arange(P)[:, None]
    j = np.arange(P)[None, :]
    return np.where(i > j, np.float32(NEG), np.float32(0.0))


def _np_dt():
    if MM_DT == mybir.dt.bfloat16:
        import ml_dtypes
        return np.dtype(ml_dtypes.bfloat16)
    return np.dtype(np.float32)


def host_inputs(x, Wqkv, Wout, core):
    """Per-core input dict (cast to the compute dtype on host)."""
    ndt = _np_dt()
    b, g = core // 2, core % 2
    xt = np.ascontiguousarray(x[b].T).astype(ndt)  # [1024, 2048]
    perm = np.concatenate([np.arange(0, DH, 2), np.arange(1, DH, 2)])
    blocks = []
    for hp in range(HPAIRS):
        hA = 8 * g + 2 * hp
        for off, do_perm in ((0, True), (D_MODEL, True), (2 * D_MODEL, False)):
            for h in (hA, hA + 1):
                rows = Wqkv[off + h * DH: off + (h + 1) * DH]
                if do_perm:
                    rows = rows[perm]
                blocks.append(rows)
    wq = np.ascontiguousarray(np.concatenate(blocks, 0).T).astype(ndt)
    wo = np.ascontiguousarray(Wout[:, 512 * g:512 * (g + 1)].T).astype(ndt)
    cos, sins = _rope_tables()
    return {"xt": xt, "wqkv": wq, "wout": wo,
            "costab": cos.astype(ndt), "sinswt": (-sins).astype(ndt),
            "masks": _masks().astype(ndt)}


_CACHE = {}


def kernel(x, Wqkv, Wout):
    from concourse.bass_utils import run_bass_kernel_spmd

    x = np.asarray(x, dtype=np.float32)
    Wqkv = np.asarray(Wqkv, dtype=np.float32)
    Wout = np.asarray(Wout, dtype=np.float32)

    if "nc" not in _CACHE:
        _CACHE["nc"] = build_program(debug=False)
    nc = _CACHE["nc"]

    in_maps = [host_inputs(x, Wqkv, Wout, c) for c in range(N_CORES)]
    res = run_bass_kernel_spmd(nc, in_maps, list(range(N_CORES))).results
    out = np.empty((B, S, D_MODEL), dtype=np.float32)
    for b in range(B):
        out[b] = res[2 * b]["out"] + res[2 * b + 1]["out"]
    return out


# revision 32
# speedup vs baseline: 1.2588x; 1.0078x over previous
"""Causal multi-head attention with RoPE on 8 Trainium2 NeuronCores.

Reference computation (fp32):
    qkv = x @ Wqkv.T ; split q,k,v ; heads 16 x 64 ; interleaved-pair RoPE on
    q,k ; causal softmax(q k^T / 8) @ v ; concat heads ; out @ Wout.T

Sharding: core c -> batch b=c//2, head-group g=c%2 (heads 8g..8g+8).
Each core computes a [2048, 1024] partial of the output projection for its
batch (contraction over its 512 head-dims); host sums core pairs.

Kernel-internal layout tricks:
  - Wqkv rows per head are permuted evens-then-odds so RoPE becomes
    block-wise (no interleaving on device). The same permutation applied to
    q and k leaves q.k^T invariant.
  - Scores are computed transposed (S^T[k, q]) so the PV matmul needs no
    transposes; PV uses a ones-augmented V (M=65) so row 64 of the PV psum
    accumulates the softmax denominator for free.
  - Causal masks are added into the scores psum by an accumulating
    identity @ mask matmul on the PE (keeps DVE free, keeps PE warm).
  - Denominators are transposed via the PE, reciprocated in fp32, and
    broadcast across partitions with a ones @ diag(recip) matmul; the
    division is one elementwise multiply per output tile.

Matmul dtype MM_DT (env): bfloat16 (default, host pre-rounds inputs),
float32r, or float32. The softmax denominator / division chain is fp32
in all modes.
"""

import math
import os
import sys

import numpy as np

sys.path.insert(0, "/opt/trn_rl_repo")

import concourse.bass as bass  # noqa: E402,F401  (re-exported for tooling)
import concourse.mybir as mybir  # noqa: E402
from concourse import bacc, tile  # noqa: E402
from concourse.masks import make_identity  # noqa: E402

D_MODEL = 1024
NUM_HEADS = 16
DH = 64
S = 2048
B = 4
THETA = 10000.0
P = 128
N_CORES = 8
F = 512  # free-dim chunk
N_SC = S // F  # 4 s-chunks
N_QT = S // P  # 16 q-tiles of 128
HPAIRS = 4  # head pairs per core
NEG = -1.0e30

MM_DT = getattr(mybir.dt, os.environ.get("MM_DT", "bfloat16"))


def build_program(debug: bool = False):
    """Build the single-core SPMD program (identical on all 8 cores)."""
    nc = bacc.Bacc("TRN2", target_bir_lowering=False, debug=debug,
                   enable_asserts=debug)
    f32 = mybir.dt.float32
    cdt = MM_DT

    xt_d = nc.dram_tensor("xt", [D_MODEL, S], cdt, kind="ExternalInput")
    wq_d = nc.dram_tensor("wqkv", [D_MODEL, 12 * P], cdt, kind="ExternalInput")
    wo_d = nc.dram_tensor("wout", [4 * P, D_MODEL], cdt, kind="ExternalInput")
    cos_d = nc.dram_tensor("costab", [P, S], cdt, kind="ExternalInput")
    sinw_d = nc.dram_tensor("sinswt", [P, S], cdt, kind="ExternalInput")
    mask_d = nc.dram_tensor("masks", [P, P], cdt, kind="ExternalInput")
    out_d = nc.dram_tensor("out", [S, D_MODEL], f32, kind="ExternalOutput")

    xt_r = xt_d.ap().rearrange("(dc p) s -> p dc s", p=P)  # [128, 8, 2048]
    wq_r = wq_d.ap().rearrange("(dc p) n -> p dc n", p=P)  # [128, 8, 1536]
    wo_r = wo_d.ap().rearrange("(hp p) e -> p hp e", p=P)  # [128, 4, 1024]


    with tile.TileContext(nc) as tc:
        with (
            tc.tile_pool(name="const", bufs=1) as const,
            tc.tile_pool(name="wq", bufs=2) as wqp,
            tc.tile_pool(name="qkv", bufs=2) as qkvp,
            tc.tile_pool(name="tmp", bufs=3) as tmpp,
            tc.tile_pool(name="outt", bufs=1) as outtp,
            tc.tile_pool(name="exp", bufs=8) as expp,
            tc.tile_pool(name="fin", bufs=3) as finp,
            tc.tile_pool(name="small", bufs=2) as smallp,
            tc.tile_pool(name="psb", bufs=4, space="PSUM") as psb,
            tc.tile_pool(name="pss", bufs=2, space="PSUM") as pss,
        ):
            # ---- constants ----
            ident = const.tile([P, P], f32)
            make_identity(nc, ident)
            identc = const.tile([P, P], cdt)
            nc.vector.tensor_copy(identc[:], ident[:])
            ones64 = const.tile([P, 64], f32)
            nc.vector.memset(ones64[:], 1.0)
            rdt = mybir.dt.float32r if MM_DT != mybir.dt.float32 else f32
            ones64r = const.tile([P, 64], rdt)
            nc.vector.tensor_copy(ones64r[:], ones64[:])
            ident4 = const.tile([P, 4, P], rdt)
            for _i4 in range(4):
                nc.vector.tensor_copy(ident4[:, _i4, :], ident[:])
            cost = const.tile([P, S], cdt)
            nc.sync.dma_start(cost[:], cos_d.ap())
            sinw = const.tile([P, S], cdt)
            nc.sync.dma_start(sinw[:], sinw_d.ap())
            maskt = const.tile([P, P], cdt)
            nc.sync.dma_start(maskt[:], mask_d.ap())
            woutt = const.tile([P, 4, D_MODEL], cdt)
            nc.sync.dma_start(woutt[:], wo_r)
            # x^T resident: [128, dchunk, s]
            xts = const.tile([P, 8, S], cdt)
            nc.sync.dma_start(xts[:], xt_r)
            # attention output (d-major), all 4 head pairs: rows=[hA|hB] dims
            outt = outtp.tile([P, HPAIRS, S], cdt)
            # softmax denominators, one tile per head pair: head A on row 0,
            # head B on row 32 (engine APs need 32-aligned start partitions)
            rs_hp = []
            for _hp in range(HPAIRS):
                _t = const.tile([P, S], f32, name=f"rs{_hp}")
                nc.vector.memset(_t[:], 1.0)
                rs_hp.append(_t)
            # reciprocals, transposed: [q-within-tile, qtile, hpair*2+h2]
            rcp = const.tile([P, N_QT, 8], f32)

            for hp in range(HPAIRS):
                _sid_qkv = nc.enter_named_scope(f"qkv{hp}", False)[0]
                whp = wqp.tile([P, 8, 3 * P], cdt)
                nc.sync.dma_start(whp[:], wq_r[:, :, hp * 3 * P:(hp + 1) * 3 * P])
                q_rot = qkvp.tile([P, S], cdt, tag="q_rot")
                k_rot = qkvp.tile([P, S], cdt, tag="k_rot")
                # V s-major + ones cols: [s-part, ktile, (vA|1|vB|1)]
                v_sb = qkvp.tile([P, N_QT, 130], cdt, tag="v_sb")
                nc.vector.tensor_copy(v_sb[:, :, 64:65], ones64[:, 0:N_QT, None])
                nc.vector.tensor_copy(v_sb[:, :, 129:130],
                                      ones64[:, 0:N_QT, None])

                for sc in range(N_SC):
                    sl = slice(sc * F, (sc + 1) * F)
                    # q and k groups (d-major); psum evac to sbuf via ACT,
                    # then RoPE on DVE in the compute dtype (2x mode for bf16)
                    for gi, dst in ((0, q_rot), (1, k_rot)):
                        ps = psb.tile([P, F], f32, tag="s1", bufs=4,
                                      name="psw")
                        for dc in range(8):
                            nc.tensor.matmul(
                                ps,
                                whp[:, dc, gi * P:(gi + 1) * P],
                                xts[:, dc, sl],
                                start=(dc == 0), stop=(dc == 7),
                            )
                        qk = tmpp.tile([P, F], cdt, tag="qk_sb")
                        nc.scalar.copy(qk[:], ps)
                        # rot = qk*cos + swap_within_head(qk)*sins
                        tcs = tmpp.tile([P, F], cdt, tag="ropetmp")
                        nc.vector.tensor_tensor(tcs[:], qk[:], cost[:, sl],
                                                mybir.AluOpType.mult)
                        for h2 in (0, 64):
                            nc.vector.tensor_tensor(
                                dst[h2:h2 + 32, sl], qk[h2 + 32:h2 + 64, :],
                                sinw[h2 + 32:h2 + 64, sl],
                                mybir.AluOpType.mult)
                            nc.vector.tensor_tensor(
                                dst[h2 + 32:h2 + 64, sl], qk[h2:h2 + 32, :],
                                sinw[h2:h2 + 32, sl], mybir.AluOpType.mult)
                        nc.vector.tensor_tensor(dst[:, sl], dst[:, sl],
                                                tcs[:], mybir.AluOpType.add)
                    # v group: d-major matmul, then PE-transpose to s-major
                    ps = psb.tile([P, F], f32, tag="s1", bufs=4,
                                  name="pswv")
                    for dc in range(8):
                        nc.tensor.matmul(
                            ps, whp[:, dc, 2 * P:3 * P],
                            xts[:, dc, sl], start=(dc == 0), stop=(dc == 7),
                        )
                    vdm = tmpp.tile([P, F], cdt, tag="vdm")
                    nc.scalar.copy(vdm[:], ps)
                    for j in range(4):
                        kt = sc * 4 + j
                        pt = pss.tile([P, F], cdt, tag="small", name="pt")
                        nc.tensor.transpose(pt[:, 0:P],
                                            vdm[:, j * P:(j + 1) * P],
                                            identc[:])
                        nc.vector.tensor_copy(v_sb[:, kt, 0:64], pt[:, 0:64])
                        nc.vector.tensor_copy(v_sb[:, kt, 65:129],
                                              pt[:, 64:128])

                nc.leave_named_scope(f"qkv{hp}", _sid_qkv, False)
                # ---- causal attention for this head pair ----
                _sid_attn = nc.enter_named_scope(f"attn{hp}", False)[0]
                for qc in range(N_SC):
                    qsl = slice(qc * F, (qc + 1) * F)
                    po = [psb.tile([P, F], f32, tag="po", bufs=2,
                                   name=f"po{h2}")
                          for h2 in range(2)]
                    nkt = 4 * qc + 4
                    LOOKAHEAD = 2

                    def emit_scores(kt):
                        lo = max(0, (kt - 4 * qc) * P)
                        diag = kt >= 4 * qc
                        sps = []
                        # head A and B score matmuls adjacent: disjoint row
                        # groups (0:64 / 64:128) run concurrently on the PE
                        for h2 in (0, 1):
                            sp = psb.tile([P, F], f32, tag="s1", bufs=4,
                                          name=f"s1_{h2}")
                            sps.append(sp)
                            base = 64 * h2
                            nc.tensor.matmul(
                                sp[:, lo:F],
                                k_rot[base:base + 64, kt * P:(kt + 1) * P],
                                q_rot[base:base + 64,
                                      qc * F + lo:(qc + 1) * F],
                                start=True, stop=not diag,
                                skip_group_check=diag,
                            )
                        if diag:
                            # strictly-upper triangle mask on the 128-wide
                            # diagonal block only
                            for h2 in (0, 1):
                                nc.tensor.matmul(
                                    sps[h2][:, lo:lo + P],
                                    identc[:], maskt[:],
                                    start=False, stop=True,
                                    skip_group_check=True,
                                )
                        return sps

                    pend = {kt: emit_scores(kt)
                            for kt in range(min(LOOKAHEAD, nkt))}
                    for kt in range(nkt):
                        lo = max(0, (kt - 4 * qc) * P)
                        if kt + LOOKAHEAD < nkt:
                            pend[kt + LOOKAHEAD] = emit_scores(kt + LOOKAHEAD)
                        sps = pend.pop(kt)
                        exs = []
                        for h2 in (0, 1):
                            ex = expp.tile([P, F], cdt)
                            if lo > 0:
                                nc.gpsimd.memset(ex[:, 0:lo], 0.0)
                            nc.scalar.activation(
                                ex[:, lo:F], sps[h2][:, lo:F],
                                mybir.ActivationFunctionType.Exp,
                                scale=1.0 / math.sqrt(DH))
                            exs.append(ex)
                        for h2 in (0, 1):
                            nc.tensor.matmul(
                                po[h2][0:65, :],
                                v_sb[:, kt, 65 * h2:65 * h2 + 65],
                                exs[h2][:],
                                start=(kt == 0), stop=(kt == nkt - 1),
                            )
                    for h2 in (0, 1):
                        nc.vector.tensor_copy(
                            outt[64 * h2:64 * h2 + 64, hp, qsl],
                            po[h2][0:64, :])
                        nc.vector.tensor_copy(rs_hp[hp][32 * h2:32 * h2 + 1,
                                                        qsl],
                                              po[h2][64:65, :])

                nc.leave_named_scope(f"attn{hp}", _sid_attn, False)
                _sid_div = nc.enter_named_scope(f"div{hp}", False)[0]
                # denominators for this head pair: transpose+reciprocal,
                # then broadcast and divide (overlaps next pair's attention)
                for j in range(N_QT):
                    pt = pss.tile([P, F], f32, tag="small", name="ptr")
                    nc.tensor.transpose(pt[:, 0:P],
                                        rs_hp[hp][:, j * P:(j + 1) * P],
                                        ident[:])
                    for h2 in (0, 1):
                        nc.vector.reciprocal(rcp[:, j, 2 * hp + h2:
                                                 2 * hp + h2 + 1],
                                             pt[:, 32 * h2:32 * h2 + 1])
                diags = {}
                for qc in range(N_SC):
                    for h2 in (0, 1):
                        h = 2 * hp + h2
                        diag4 = smallp.tile([P, 4, P], rdt, tag="diag", bufs=8)
                        nc.vector.tensor_tensor(
                            diag4[:], ident4[:],
                            rcp[:, 4 * qc:4 * qc + 4, h:h + 1].to_broadcast(
                                (P, 4, P)),
                            mybir.AluOpType.mult)
                        diags[(qc, h2)] = diag4
                for qc in range(N_SC):
                    qsl = slice(qc * F, (qc + 1) * F)
                    for h2 in (0, 1):
                        pbcw = pss.tile([P, F], f32, tag="small",
                                        name="pbcw")
                        pbc = pbcw[:, 0:F]
                        nc.tensor.matmul(pbc[0:64, :], ones64r[:, 0:64],
                                         diags[(qc, h2)][:],
                                         start=True, stop=True)
                        nc.vector.tensor_tensor(
                            outt[64 * h2:64 * h2 + 64, hp, qsl],
                            outt[64 * h2:64 * h2 + 64, hp, qsl],
                            pbc[0:64, :], mybir.AluOpType.mult)

                nc.leave_named_scope(f"div{hp}", _sid_div, False)
            # ---- output projection: natural [s, e] partial ----
            _sid_proj = nc.enter_named_scope("proj", False)[0]
            for ec in range(2):
                esl = slice(ec * F, (ec + 1) * F)
                for st in range(N_QT):
                    pf = psb.tile([P, F], f32, tag="s1", bufs=4,
                                  name="pfw")
                    for hp in range(HPAIRS):
                        nc.tensor.matmul(
                            pf, outt[:, hp, st * P:(st + 1) * P],
                            woutt[:, hp, esl],
                            start=(hp == 0), stop=(hp == 3),
                        )
                    fo = finp.tile([P, F], f32)
                    if st % 2 == 0:
                        nc.scalar.copy(fo[:], pf)
                    else:
                        nc.vector.tensor_copy(fo[:], pf)
                    nc.sync.dma_start(
                        out_d.ap()[st * P:(st + 1) * P, esl], fo[:])
            nc.leave_named_scope("proj", _sid_proj, False)

    nc.compile()
    return nc


def _rope_tables():
    k = np.arange(DH // 2, dtype=np.float64)
    invf = THETA ** (-2.0 * k / DH)
    pos = np.arange(S, dtype=np.float64)
    ang = invf[:, None] * pos[None, :]  # [32, S]
    cos32 = np.cos(ang)
    sin32 = np.sin(ang)
    cos = np.tile(cos32, (4, 1)).astype(np.float32)          # [128, S]
    sins = np.concatenate([-sin32, sin32, -sin32, sin32], 0).astype(np.float32)
    return cos, sins


def _masks():
    i = np.arange(P)[:, None]
    j = np.arange(P)[None, :]
    return np.where(i > j, np.float32(NEG), np.float32(0.0))


def _np_dt():
    if MM_DT == mybir.dt.bfloat16:
        import ml_dtypes
        return np.dtype(ml_dtypes.bfloat16)
    return np.dtype(np.float32)


def host_inputs(x, Wqkv, Wout, core):
    """Per-core input dict (cast to the compute dtype on host)."""
    ndt = _np_dt()
    b, g = core // 2, core % 2
    xt = np.ascontiguousarray(x[b].T).astype(ndt)  # [1024, 2048]
    perm = np.concatenate([np.arange(0, DH, 2), np.arange(1, DH, 2)])
    blocks = []
    for hp in range(HPAIRS):
        hA = 8 * g + 2 * hp
        for off, do_perm in ((0, True), (D_MODEL, True), (2 * D_MODEL, False)):
            for h in (hA, hA + 1):
                rows = Wqkv[off + h * DH: off + (h + 1) * DH]
                if do_perm:
                    rows = rows[perm]
                blocks.append(rows)
    wq = np.ascontiguousarray(np.concatenate(blocks, 0).T).astype(ndt)
    wo = np.ascontiguousarray(Wout[:, 512 * g:512 * (g + 1)].T).astype(ndt)
    cos, sins = _rope_tables()
    return {"xt": xt, "wqkv": wq, "wout": wo,
            "costab": cos.astype(ndt), "sinswt": (-sins).astype(ndt),
            "masks": _masks().astype(ndt)}


_CACHE = {}


def kernel(x, Wqkv, Wout):
    from concourse.bass_utils import run_bass_kernel_spmd

    x = np.asarray(x, dtype=np.float32)
    Wqkv = np.asarray(Wqkv, dtype=np.float32)
    Wout = np.asarray(Wout, dtype=np.float32)

    if "nc" not in _CACHE:
        _CACHE["nc"] = build_program(debug=False)
    nc = _CACHE["nc"]

    in_maps = [host_inputs(x, Wqkv, Wout, c) for c in range(N_CORES)]
    res = run_bass_kernel_spmd(nc, in_maps, list(range(N_CORES))).results
    out = np.empty((B, S, D_MODEL), dtype=np.float32)
    for b in range(B):
        out[b] = res[2 * b]["out"] + res[2 * b + 1]["out"]
    return out
